# revision 23
# baseline (speedup 1.0000x reference)
"""Trainium2 Bass kernel for nn_ASTGATClassifier (3-layer GAT + BN + ELU + pool + MLP).

v2 strategy (8 NeuronCores, SPMD single program), built around the TimelineSim
cost model's collective pricing (cost = output bytes):

  - Edges are SRC-partitioned: core c owns edges whose src node lies in its
    contiguous node shard.  x = h @ W is computed locally; no AllGather of
    features.  GAT softmax division is deferred past the collective:
    out[dst] = (sum_e exp_e * x[src_e]) / (sum_e exp_e), so each core
    scatter-adds *partial* numerators+denominators for ALL destination nodes
    into a DRAM accumulator, and a chunked bf16 ReduceScatter(add) delivers
    each core its own dst shard (RS output is 1/8 the bytes of the AllGather
    the baseline used -> ~27x cheaper per moved byte under the cost model).
  - Attention-destination terms al_d travel via a tiny AllGather of [N2,8]
    plus a DRAM->DRAM expansion into 256B rows for the per-edge gather.
  - Scatter-add is one-hot matmuls: edges sorted by (chunk-major) global dst
    row; per 128-row dst window, [128e,516] rhs (exp-scaled x ++ exp cols)
    against an is_equal one-hot lhsT accumulates into fp32 PSUM; windows flush
    to a bf16 stage and batched region DMAs feed the ReduceScatter chunks,
    which overlap the next chunk's edge processing.
  - Cross-core program uniformity: per-window slot counts are ceil16 of the
    max edge count over cores; (chunk, ald-half) runs padded to multiples of
    128 so gather calls stay int16-indexable and tile-aligned.  All schedule
    structure is identical across cores; only index/dstloc DATA differs.
  - Post-RS: rowwise divide, BN stats via ones-matmuls + AllReduce, transpose
    + affine + ELU into hTe (lhsT for the next layer).  Pooling + classifier
    as in v1 (masked per-core segment reduces + AllReduce, replicated MLP).
"""

import sys

sys.path.insert(0, "/opt/trn_rl_repo")

import numpy as np
import ml_dtypes

N_NODES = 50000
N_EDGES = 400000
N_GRAPHS = 256
NUM_TYPES = 200
EMB = 64
HID = 128
HEADS = 4
GDIM = 256
NUM_CLASSES = 20
EPS = 1e-5
NEG = 0.2
EPS_DEN = 1e-20

NC = 8
NSH = N_NODES // NC          # 6250 nodes per core
NBLK = 49                    # node blocks per core (49*128 = 6272)
N2 = NBLK * 128              # padded shard
NTOT = NC * N2               # 50176
K_CH = 7                     # RS chunks
CH = N2 // K_CH              # 896 rows per (chunk, core) region
RB = CH // 128               # 7 windows per region
NW = K_CH * NC * RB          # 392 windows
ALD_HALF = (NC // 2) * N2    # 25088

# per-layer config: (IN, OC, H, EW(gather row cols), AC(accum cols))
LCFG = [
    (EMB, 512, 4, 640, 516),
    (512, 512, 4, 640, 516),
    (512, 128, 1, 256, 129),
]
SPAN_SLOTS = 1792            # max slots per gather call (14 tiles)

BF16 = ml_dtypes.bfloat16

_CACHE = {}


def _wrap_idx(idx):
    """int16 gather index layout: [128, n/16]; idx j at [j%16, j//16], tiled x8."""
    n = len(idx)
    assert n % 16 == 0
    a = np.asarray(idx, dtype=np.int16).reshape(n // 16, 16).T
    return np.tile(a, (8, 1))


def preprocess(x, edge_index, depth, batch):
    """Host-side index preprocessing -> per-core blobs + uniform schedule."""
    x = np.asarray(x).astype(np.int64)
    ei = np.asarray(edge_index).astype(np.int64)
    batch = np.asarray(batch).astype(np.int64)
    loop = np.arange(N_NODES, dtype=np.int64)
    src = np.concatenate([ei[0], loop])
    dst = np.concatenate([ei[1], loop])

    # destination-side row mappings (global)
    oc = dst // NSH
    locd = dst - oc * NSH
    kch = locd // CH
    arow = kch * (NC * CH) + oc * CH + (locd - kch * CH)   # accum row (chunk-major)
    wind = arow // 128
    d128 = arow % 128
    aldrow = oc * N2 + locd                                 # ald table row
    half = (oc >= NC // 2).astype(np.int64)
    aldidx = aldrow - half * ALD_HALF

    core_of_src = src // NSH
    percore = []
    cnts = np.zeros((NC, NW), dtype=np.int64)
    for c in range(NC):
        m = core_of_src == c
        sl = (src[m] - c * NSH).astype(np.int64)
        wc, ac, dc, aic = wind[m], arow[m], d128[m], aldidx[m]
        order = np.argsort(ac, kind="stable")
        sl, wc, dc, aic = sl[order], wc[order], dc[order], aic[order]
        percore.append((sl, wc, dc, aic))
        cnts[c] = np.bincount(wc, minlength=NW)

    slots_w = ((cnts.max(axis=0) + 15) // 16 * 16).astype(np.int64)

    # half of a window (uniform): windows ordered (k, oc, b); oc = (w//RB) % NC
    w_half = ((np.arange(NW) // RB) % NC) >= (NC // 2)

    # build slot stream: windows in order; pad each (k, half) run to %128
    slot_w = []          # per-slot window id (-1 = run pad)
    ws_start = np.zeros(NW, dtype=np.int64)
    run_bounds = []      # (slot_lo, slot_hi, half) per run
    pos = 0
    for k in range(K_CH):
        for hf in range(2):
            run_lo = pos
            w0 = k * NC * RB + hf * (NC // 2) * RB
            for w in range(w0, w0 + (NC // 2) * RB):
                ws_start[w] = pos
                slot_w.extend([w] * int(slots_w[w]))
                pos += int(slots_w[w])
            r = (-pos) % 128
            slot_w.extend([-1] * r)
            pos += r
            run_bounds.append((run_lo, pos, hf))
    S = pos
    assert S % 128 == 0
    slot_w = np.asarray(slot_w, dtype=np.int64)
    n_tiles = S // 128

    # spans: cut runs into <= SPAN_SLOTS pieces (128-aligned)
    spans = []
    for (lo, hi, hf) in run_bounds:
        p = lo
        while p < hi:
            n = min(SPAN_SLOTS, hi - p)
            spans.append((p, n, hf))
            p += n

    # pairs: per tile, windows overlapping its slot range
    ws_end = ws_start + slots_w
    pairs = []            # (tile, w, first, last, pair_col)
    pairs_of_tile = [[] for _ in range(n_tiles)]
    first_pair = {}
    last_pair = {}
    for w in range(NW):
        if slots_w[w] == 0:
            continue
        t0 = int(ws_start[w] // 128)
        t1 = int((ws_end[w] - 1) // 128)
        for t in range(t0, t1 + 1):
            pc = len(pairs)
            pairs.append([t, w, t == t0, t == t1])
            pairs_of_tile[t].append(pc)
    NPAIR = len(pairs)

    # per-core blobs
    blobs = []
    for c in range(NC):
        sl, wc, dc, aic = percore[c]
        xidx = np.zeros(S, dtype=np.int64)
        aidx = np.zeros(S, dtype=np.int64)
        dloc = np.full(S, -1.0, dtype=np.float32)
        # per window: this core's edges occupy the first cnt slots
        offs = ws_start[np.searchsorted(np.arange(NW), wc)]  # = ws_start[wc]
        offs = ws_start[wc]
        within = np.zeros(len(wc), dtype=np.int64)
        # position within window = running count per window (wc sorted)
        uw, uidx, ucnt = np.unique(wc, return_index=True, return_counts=True)
        for u, i0, cc in zip(uw, uidx, ucnt):
            within[i0 : i0 + cc] = np.arange(cc)
        slot = offs + within
        xidx[slot] = sl
        aidx[slot] = aic
        dloc[slot] = dc.astype(np.float32)
        # dstloc blob per pair
        dl = np.full((128, NPAIR), -1.0, dtype=np.float32)
        for pc, (t, w, fi, la) in enumerate(pairs):
            seg = dloc[t * 128 : (t + 1) * 128].copy()
            m = slot_w[t * 128 : (t + 1) * 128] != w
            seg[m] = -1.0
            dl[:, pc] = seg
        blobs.append(
            dict(
                xidx=_wrap_idx(xidx),
                aldidx=_wrap_idx(aidx),
                dstloc=dl,
            )
        )

    # emb gather idx + depth rows per core (layer-0 prolog)
    for c in range(NC):
        ids = np.zeros(N2, dtype=np.int64)
        ids[:NSH] = x[c * NSH : (c + 1) * NSH]
        blobs[c]["emb_idx"] = _wrap_idx(ids)
        dr = np.zeros((1, N2), dtype=np.float32)
        dr[0, :NSH] = np.asarray(depth, dtype=np.float32)[c * NSH : (c + 1) * NSH]
        blobs[c]["depth_row"] = dr

    # pooling segments (per baseline)
    counts = np.bincount(batch, minlength=N_GRAPHS)
    starts = np.concatenate([[0], np.cumsum(counts)])
    segs = []
    for cc in range(NC):
        lo_n, hi_n = cc * NSH, (cc + 1) * NSH
        lst = []
        for g in range(N_GRAPHS):
            a, bnd = starts[g], starts[g + 1]
            aa, bb = max(a, lo_n), min(bnd, hi_n)
            if aa < bb:
                lst.append((int(aa - lo_n), int(bb - lo_n), int(g), float(1.0 / max(counts[g], 1))))
        segs.append(lst)
    for c in range(NC):
        m8 = np.zeros((128, NC), dtype=np.float32)
        m8[:, c] = 1.0
        m8n = np.where(m8 > 0, 0.0, -1e30).astype(np.float32)
        blobs[c]["mask8"] = m8
        blobs[c]["mask8n"] = m8n

    sched = dict(
        slots_w=slots_w, ws_start=ws_start, ws_end=ws_end, spans=spans,
        pairs=pairs, pairs_of_tile=pairs_of_tile, n_tiles=n_tiles, S=S,
        NPAIR=NPAIR, segs=segs,
    )
    return dict(sched=sched, blobs=blobs)


def build_param_blobs(p):
    """Host-side parameter layout transforms (bf16 casts, folds, transposes)."""
    f32 = np.float32
    out = {}

    def fold_a(W, a_s, a_d, heads, c):
        W3 = W.reshape(heads, c, -1)
        As = np.einsum("hck,hc->kh", W3, a_s).astype(f32)
        Ad = np.einsum("hck,hc->kh", W3, a_d).astype(f32)
        return np.concatenate([As, Ad], axis=1)  # [IN, 2H]

    out["w0x"] = np.ascontiguousarray(p["W0"].T).astype(BF16)
    out["w0al"] = fold_a(p["W0"], p["as0"], p["ad0"], HEADS, HID).astype(BF16)
    out["w1x"] = np.ascontiguousarray(p["W1"].T).astype(BF16)
    out["w1al"] = fold_a(p["W1"], p["as1"], p["ad1"], HEADS, HID).astype(BF16)
    out["w2x"] = np.ascontiguousarray(p["W2"].T).astype(BF16)
    out["w2al"] = fold_a(p["W2"], p["as2"], p["ad2"], 1, GDIM // 2).astype(BF16)
    out["emb_t"] = np.asarray(p["emb_table"], dtype=f32)
    out["dw_row"] = np.asarray(p["depth_w"], dtype=f32).reshape(1, EMB)
    out["db_row"] = np.asarray(p["depth_b"], dtype=f32).reshape(1, EMB)
    for l, (g, be) in enumerate([(p["g0"], p["be0"]), (p["g1"], p["be1"]), (p["g2"], p["be2"])]):
        out[f"gam{l}"] = np.asarray(g, dtype=f32).reshape(1, -1)
        out[f"bet{l}"] = np.asarray(be, dtype=f32).reshape(1, -1)
    out["cw1t"] = np.ascontiguousarray(p["cw1"].T).astype(f32)
    out["cb1c"] = np.asarray(p["cb1"], dtype=f32).reshape(2, 128).T.copy()
    out["cw2t"] = np.ascontiguousarray(p["cw2"].T).astype(f32)
    out["cb2c"] = np.asarray(p["cb2"], dtype=f32).reshape(NUM_CLASSES, 1)
    out["iotab"] = np.tile(np.arange(128, dtype=f32)[None, :], (128, 1)).astype(BF16)
    out["iden_f"] = np.eye(128, dtype=f32)
    out["iden_b"] = np.eye(128).astype(BF16)
    out["ones_b"] = np.ones((128, 1), dtype=BF16)
    out["ones_r"] = np.ones((1, 128), dtype=f32)
    return out


def build_nc(pre):
    """Trace the full SPMD bass program (structure from `pre['sched']`)."""
    import concourse.bacc as bacc
    import concourse.bass as bass
    import concourse.mybir as mybir
    import concourse.tile as tile
    from concourse.library_config import mlp
    from contextlib import ExitStack

    dt = mybir.dt
    ALU = mybir.AluOpType
    ACTF = mybir.ActivationFunctionType
    AXX = mybir.AxisListType.X

    sch = pre["sched"]
    spans = sch["spans"]
    pairs = sch["pairs"]
    pairs_of_tile = sch["pairs_of_tile"]
    n_tiles = sch["n_tiles"]
    S = sch["S"]
    NPAIR = sch["NPAIR"]
    slots_w = sch["slots_w"]
    segs = sch["segs"]

    nc = bacc.Bacc("TRN2", target_bir_lowering=False, debug=False, num_devices=NC)

    b0 = pre["blobs"][0]
    EIN = {}

    def ein(name, arr_like, dtyp):
        EIN[name] = nc.dram_tensor(name, list(arr_like.shape), dtyp, kind="ExternalInput").ap()
        return EIN[name]

    i_xidx = ein("xidx", b0["xidx"], dt.int16)
    i_aldidx = ein("aldidx", b0["aldidx"], dt.int16)
    i_dstloc = ein("dstloc", b0["dstloc"], dt.float32)
    i_embidx = ein("emb_idx", b0["emb_idx"], dt.int16)
    i_depth = ein("depth_row", b0["depth_row"], dt.float32)
    i_mask8 = ein("mask8", b0["mask8"], dt.float32)
    i_mask8n = ein("mask8n", b0["mask8n"], dt.float32)
    P = {}
    P["w0x"] = ein("w0x", np.zeros((EMB, 512)), dt.bfloat16)
    P["w0al"] = ein("w0al", np.zeros((EMB, 8)), dt.bfloat16)
    P["w1x"] = ein("w1x", np.zeros((512, 512)), dt.bfloat16)
    P["w1al"] = ein("w1al", np.zeros((512, 8)), dt.bfloat16)
    P["w2x"] = ein("w2x", np.zeros((512, 128)), dt.bfloat16)
    P["w2al"] = ein("w2al", np.zeros((512, 2)), dt.bfloat16)
    P["emb_t"] = ein("emb_t", np.zeros((NUM_TYPES, EMB)), dt.float32)
    P["dw_row"] = ein("dw_row", np.zeros((1, EMB)), dt.float32)
    P["db_row"] = ein("db_row", np.zeros((1, EMB)), dt.float32)
    for l, ocl in [(0, 512), (1, 512), (2, 128)]:
        P[f"gam{l}"] = ein(f"gam{l}", np.zeros((1, ocl)), dt.float32)
        P[f"bet{l}"] = ein(f"bet{l}", np.zeros((1, ocl)), dt.float32)
    P["cw1t"] = ein("cw1t", np.zeros((GDIM, GDIM)), dt.float32)
    P["cb1c"] = ein("cb1c", np.zeros((128, 2)), dt.float32)
    P["cw2t"] = ein("cw2t", np.zeros((GDIM, NUM_CLASSES)), dt.float32)
    P["cb2c"] = ein("cb2c", np.zeros((NUM_CLASSES, 1)), dt.float32)
    P["iotab"] = ein("iotab", np.zeros((128, 128)), dt.bfloat16)
    P["iden_f"] = ein("iden_f", np.zeros((128, 128)), dt.float32)
    P["iden_b"] = ein("iden_b", np.zeros((128, 128)), dt.bfloat16)
    P["ones_b"] = ein("ones_b", np.zeros((128, 1)), dt.bfloat16)
    P["ones_r"] = ein("ones_r", np.zeros((1, 128)), dt.float32)

    out_dram = nc.dram_tensor("out", [N_GRAPHS, NUM_CLASSES], dt.float32, kind="ExternalOutput").ap()

    with tile.TileContext(nc) as tc, ExitStack() as stk:
        nc.gpsimd.load_library(mlp)
        sb = stk.enter_context(tc.tile_pool(name="sb", bufs=2))
        sb1 = stk.enter_context(tc.tile_pool(name="sb1", bufs=1))
        ps = stk.enter_context(tc.tile_pool(name="ps", bufs=2, space="PSUM"))
        dram = stk.enter_context(tc.tile_pool(name="dram", bufs=1, space="DRAM"))

        def load_sb(ap, shape, dtyp, tag, pool=sb1):
            t = pool.tile(shape, dtyp, tag=tag)
            nc.sync.dma_start(t[:], ap[:, :])
            return t

        iota_sb = load_sb(P["iotab"], [128, 128], dt.bfloat16, "iota")
        idenb_sb = load_sb(P["iden_b"], [128, 128], dt.bfloat16, "idenb")
        idenf_sb = load_sb(P["iden_f"], [128, 128], dt.float32, "idenf")
        onesb_sb = load_sb(P["ones_b"], [128, 1], dt.bfloat16, "onesb")
        onesr_sb = load_sb(P["ones_r"], [1, 128], dt.float32, "onesr")
        dw_sb = load_sb(P["dw_row"], [1, EMB], dt.float32, "dwrow")
        db_sb = load_sb(P["db_row"], [1, EMB], dt.float32, "dbrow")
        xidx_sb = load_sb(i_xidx, [128, S // 16], dt.int16, "xidx")
        aldidx_sb = load_sb(i_aldidx, [128, S // 16], dt.int16, "aldidx")
        dstloc_sb = load_sb(i_dstloc, [128, NPAIR], dt.float32, "dstloc")

        wx_sb = {}
        wal_sb = {}
        for l, (IN, OC, H, EW, AC) in enumerate(LCFG):
            nch_in = (IN + 127) // 128
            wx_sb[l] = []
            wal_sb[l] = []
            wxn, waln = f"w{l}x", f"w{l}al"
            for k in range(nch_in):
                kp = min(IN - k * 128, 128)
                tx = sb1.tile([kp, OC], dt.bfloat16, tag=f"wx{l}_{k}")
                nc.sync.dma_start(tx[:], P[wxn][k * 128 : k * 128 + kp, :])
                wx_sb[l].append(tx)
                ta = sb1.tile([kp, 2 * H], dt.bfloat16, tag=f"wal{l}_{k}")
                nc.sync.dma_start(ta[:], P[waln][k * 128 : k * 128 + kp, :])
                wal_sb[l].append(ta)

        # persistent transposed features (lhsT for x_phase; final = pooling input)
        hTe = sb1.tile([128, 4 * N2], dt.bfloat16, tag="hTe")

        # DRAM tensors
        x_tbl = dram.tile([N2, 640], dt.bfloat16, tag="x_tbl")
        ald_in = dram.tile([N2, 8], dt.bfloat16, tag="ald_in")
        ald_fulls = [
            dram.tile([NTOT, 8], dt.bfloat16, tag=f"ald_full{l}", addr_space="Shared", name=f"ald_full{l}")
            for l in range(3)
        ]
        ald_tbl = dram.tile([NTOT, 128], dt.bfloat16, tag="ald_tbl")
        accA = dram.tile([NTOT, 516], dt.bfloat16, tag="accA")
        accC = dram.tile([NTOT, 129], dt.bfloat16, tag="accC")
        rsA = dram.tile([N2, 516], dt.bfloat16, tag="rsA")
        rsC = dram.tile([N2, 129], dt.bfloat16, tag="rsC")

        # =========================================================
        # Layer-0 prolog: hTe[0:64] = (emb + depth-proj)^T
        # =========================================================
        embidx_sb = load_sb(i_embidx, [128, N2 // 16], dt.int16, "embidx")
        emb_g = sb.tile([128, NBLK * EMB], dt.float32, tag="xg", name="emb_g")
        nc.gpsimd.dma_gather(
            emb_g[:].rearrange("p (t w) -> p t w", w=EMB),
            P["emb_t"][:, :],
            embidx_sb[:],
            N2, N2, EMB, single_packet=False,
        )
        for nt in range(NBLK):
            dr_t = sb.tile([1, 128], dt.float32, tag="dr", bufs=2)
            nc.sync.dma_start(dr_t[:], i_depth[0:1, nt * 128 : (nt + 1) * 128])
            ps_t = ps.tile([EMB, 128], dt.float32, tag="med")
            nc.tensor.matmul(out=ps_t[:], lhsT=dw_sb[:], rhs=dr_t[:], start=True, stop=False)
            nc.tensor.matmul(out=ps_t[:], lhsT=db_sb[:], rhs=onesr_sb[:], start=False, stop=False)
            nc.tensor.matmul(
                out=ps_t[:],
                lhsT=emb_g[:, nt * EMB : (nt + 1) * EMB],
                rhs=idenf_sb[:],
                is_transpose=True,
                start=False,
                stop=True,
            )
            nc.vector.tensor_copy(hTe[0:EMB, nt * 128 : (nt + 1) * 128], ps_t[:])

        # =========================================================
        # per-layer phases
        # =========================================================
        def x_phase(l):
            IN, OC, H, EW, AC = LCFG[l]
            nch = (IN + 127) // 128

            def lhs(k, nt):
                kp = min(IN - k * 128, 128)
                return hTe[0:kp, k * N2 + nt * 128 : k * N2 + (nt + 1) * 128]

            # pass A: attention rows (al_s kept in SBUF; al_d staged + AllGather)
            als_sb = sb.tile([128, NBLK * 4], dt.float32, tag="als", bufs=1)
            alds = sb.tile([128, NBLK * 8], dt.bfloat16, tag="alds", bufs=1)
            nc.vector.memset(alds[:], 0)
            for nt in range(NBLK):
                ps_al = ps.tile([128, 8], dt.float32, tag="small")
                for k in range(nch):
                    nc.tensor.matmul(out=ps_al[:, 0 : 2 * H], lhsT=lhs(k, nt), rhs=wal_sb[l][k][:],
                                     start=(k == 0), stop=(k == nch - 1))
                nc.vector.tensor_copy(als_sb[:, nt * 4 : nt * 4 + H], ps_al[:, 0:H])
                nc.vector.tensor_copy(alds[:, nt * 8 : nt * 8 + H], ps_al[:, H : 2 * H])
            nc.sync.dma_start(
                ald_in[:, :].rearrange("(t p) w -> p t w", p=128), alds[:].rearrange("p (t w) -> p t w", w=8)
            )
            nc.gpsimd.collective_compute(
                "AllGather", ALU.bypass, ins=[ald_in[:, :]], outs=[ald_fulls[l][:, :]],
                replica_groups=[list(range(NC))],
            )
            nc.sync.dma_start(ald_tbl[:, 0:8], ald_fulls[l][:, :])

            # pass B: x rows -> x_tbl (overlaps AllGather)
            for nt in range(NBLK):
                ps_x = ps.tile([128, 512], dt.float32, tag="win", bufs=2)
                for k in range(nch):
                    nc.tensor.matmul(out=ps_x[:, 0:OC], lhsT=lhs(k, nt), rhs=wx_sb[l][k][:],
                                     start=(k == 0), stop=(k == nch - 1))
                xa_t = sb.tile([128, EW], dt.bfloat16, tag="xa_t", bufs=2)
                nc.scalar.activation(xa_t[:, 0:OC], ps_x[:, 0:OC], ACTF.Copy)
                nc.vector.tensor_copy(xa_t[:, OC : OC + H], als_sb[:, nt * 4 : nt * 4 + H])
                nc.sync.dma_start(x_tbl[nt * 128 : (nt + 1) * 128, 0:EW], xa_t[:])

        def edge_phase(l, pass1_chunk):
            IN, OC, H, EW, AC = LCFG[l]
            acc = accA if l < 2 else accC
            ST = SPAN_SLOTS // 128
            open_ps = {}
            stage = {"t": None}
            flush_cnt = [0]

            def flush(w, psw):
                ps_o, ps_d = psw
                r, b = w // RB, w % RB
                if b == 0:
                    stage["t"] = sb.tile([128, RB * AC], dt.bfloat16, tag="stage", bufs=2, name="stage_t")
                dstc = stage["t"][:, b * AC : b * AC + OC]
                nc.scalar.activation(dstc, ps_o[:, 0:OC], ACTF.Copy)
                nc.vector.tensor_copy(stage["t"][:, b * AC + OC : (b + 1) * AC], ps_d[:, 0:H])
                flush_cnt[0] += 1
                if b == RB - 1:
                    k, oc = r // NC, r % NC
                    rows0 = k * (NC * CH) + oc * CH
                    nc.sync.dma_start(
                        acc[rows0 : rows0 + CH, :].rearrange("(t p) w -> p t w", p=128),
                        stage["t"][:].rearrange("p (t w) -> p t w", w=AC),
                    )
                    if oc == NC - 1:
                        rs_out = rsA if l < 2 else rsC
                        nc.gpsimd.collective_compute(
                            "ReduceScatter", ALU.add,
                            ins=[acc[k * NC * CH : (k + 1) * NC * CH, :]],
                            outs=[rs_out[k * CH : (k + 1) * CH, :]],
                            replica_groups=[list(range(NC))],
                        )

            pair_i = 0
            for (s0, ns, hf) in spans:
                t0 = s0 // 128
                nt = ns // 128
                xg = sb.tile([128, ST * EW], dt.bfloat16, tag="xg", bufs=2)
                xg3 = xg[:, 0 : nt * EW].rearrange("p (t w) -> p t w", w=EW)
                nc.gpsimd.dma_gather(
                    xg3, x_tbl[:, 0:EW], xidx_sb[:, s0 // 16 : (s0 + ns) // 16],
                    ns, ns, EW, elem_step=640, single_packet=False,
                )
                aldg = sb.tile([128, ST * 128], dt.bfloat16, tag="aldg", bufs=2)
                aldg3 = aldg[:, 0 : nt * 128].rearrange("p (t w) -> p t w", w=128)
                nc.gpsimd.dma_gather(
                    aldg3, ald_tbl[hf * ALD_HALF : hf * ALD_HALF + ALD_HALF, :],
                    aldidx_sb[:, s0 // 16 : (s0 + ns) // 16],
                    ns, ns, 128, single_packet=False,
                )
                # logits -> exp over the span
                zt = sb.tile([128, ST * 4], dt.float32, tag="zt", bufs=3)
                nc.vector.tensor_tensor(
                    out=zt[:, 0 : nt * H].rearrange("p (t h) -> p t h", h=H),
                    in0=xg3[:, :, OC : OC + H], in1=aldg3[:, :, 0:H], op=ALU.add,
                )
                z2 = sb.tile([128, ST * 4], dt.float32, tag="z2", bufs=3)
                nc.vector.tensor_scalar(out=z2[:, 0 : nt * H], in0=zt[:, 0 : nt * H],
                                        scalar1=NEG, scalar2=None, op0=ALU.mult)
                nc.vector.tensor_tensor(out=z2[:, 0 : nt * H], in0=zt[:, 0 : nt * H],
                                        in1=z2[:, 0 : nt * H], op=ALU.max)
                ex = sb.tile([128, ST * 4], dt.float32, tag="ex", bufs=3)
                nc.scalar.activation(ex[:, 0 : nt * H], z2[:, 0 : nt * H], ACTF.Exp)
                exb = sb.tile([128, ST * 4], dt.bfloat16, tag="exb", bufs=3)
                nc.vector.tensor_copy(exb[:, 0 : nt * H], ex[:, 0 : nt * H])
                for tl in range(nt):
                    t = t0 + tl
                    plist = pairs_of_tile[t]
                    if not plist:
                        continue
                    xgs = sb.tile([128, 512], dt.bfloat16, tag="xgs", bufs=6)
                    for h in range(H):
                        nc.vector.tensor_scalar(
                            out=xgs[:, h * 128 : (h + 1) * 128],
                            in0=xg3[:, tl, h * 128 : (h + 1) * 128],
                            scalar1=ex[:, tl * H + h : tl * H + h + 1],
                            scalar2=None, op0=ALU.mult,
                        )
                    for pc in plist:
                        _, w, first, last = pairs[pc]
                        oh = sb.tile([128, 128], dt.bfloat16, tag="oh", bufs=8)
                        nc.vector.tensor_scalar(
                            out=oh[:], in0=iota_sb[:], scalar1=dstloc_sb[:, pc : pc + 1],
                            scalar2=None, op0=ALU.is_equal,
                        )
                        pair_i += 1
                        if first:
                            open_ps[w] = (
                                ps.tile([128, 512], dt.float32, tag="win", bufs=2, name="ps_win"),
                                ps.tile([128, 8], dt.float32, tag="small", name="ps_wd"),
                            )
                        ps_o, ps_d = open_ps[w]
                        nc.tensor.matmul(out=ps_o[:, 0:OC], lhsT=oh[:], rhs=xgs[:, 0:OC],
                                         start=first, stop=last)
                        nc.tensor.matmul(out=ps_d[:, 0:H], lhsT=oh[:],
                                         rhs=exb[:, tl * H : (tl + 1) * H],
                                         start=first, stop=last)
                        if last:
                            flush(w, open_ps.pop(w))

            # empty windows (no pairs anywhere): zero their stage slots via a
            # dedicated zero tile (rare; usually none)
            assert not open_ps

        def make_pass1(l, ps_s, ps_q):
            IN, OC, H, EW, AC = LCFG[l]
            rs_out = rsA if l < 2 else rsC
            nch = OC // 128

            def pass1_chunk(kc):
                for nt in range(kc * RB, (kc + 1) * RB):
                    if nt % RB == 0 or nt % 2 == 0:
                        pass
                    ld = sb.tile([128, AC], dt.bfloat16, tag="ld", bufs=3, name="ld_t")
                    nc.sync.dma_start(ld[:, 0:AC], rs_out[nt * 128 : (nt + 1) * 128, :])
                    blk = ld[:, 0:AC]
                    d4 = sb.tile([128, 4], dt.float32, tag="d4", bufs=2)
                    nc.vector.tensor_scalar(out=d4[:, 0:H], in0=blk[:, OC : OC + H],
                                            scalar1=EPS_DEN, scalar2=None, op0=ALU.add)
                    r4 = sb.tile([128, 4], dt.float32, tag="r4", bufs=2)
                    nc.vector.reciprocal(r4[:, 0:H], d4[:, 0:H])
                    hblk = sb.tile([128, 512], dt.bfloat16, tag="hblk", bufs=2)
                    for h in range(H):
                        nc.vector.tensor_scalar(
                            out=hblk[:, h * 128 : (h + 1) * 128] if H > 1 else hblk[:, 0:OC],
                            in0=blk[:, h * 128 : (h + 1) * 128] if H > 1 else blk[:, 0:OC],
                            scalar1=r4[:, h : h + 1], scalar2=None, op0=ALU.mult,
                        )
                    sqb = sb.tile([128, 512], dt.bfloat16, tag="sqb", bufs=1)
                    nc.vector.tensor_tensor(out=sqb[:, 0:OC], in0=hblk[:, 0:OC], in1=hblk[:, 0:OC], op=ALU.mult)
                    nc.tensor.matmul(out=ps_s[:], lhsT=onesb_sb[:], rhs=hblk[:, 0:OC],
                                     start=(nt == 0), stop=(nt == NBLK - 1))
                    nc.tensor.matmul(out=ps_q[:], lhsT=onesb_sb[:], rhs=sqb[:, 0:OC],
                                     start=(nt == 0), stop=(nt == NBLK - 1))
                    for kk in range(nch):
                        ps_t = ps.tile([128, 128], dt.bfloat16, tag="med")
                        nc.tensor.matmul(out=ps_t[:], lhsT=hblk[:, kk * 128 : (kk + 1) * 128],
                                         rhs=idenb_sb[:], is_transpose=True, start=True, stop=True)
                        if (nt + kk) % 2 == 0:
                            nc.scalar.activation(hTe[:, kk * N2 + nt * 128 : kk * N2 + (nt + 1) * 128], ps_t[:], ACTF.Copy)
                        else:
                            nc.vector.tensor_copy(hTe[:, kk * N2 + nt * 128 : kk * N2 + (nt + 1) * 128], ps_t[:])

            return pass1_chunk

        def post_phase(l, ps_s, ps_q):
            IN, OC, H, EW, AC = LCFG[l]
            nch = OC // 128
            # BN stats AllReduce -> affine coeffs
            gam_t = sb.tile([1, OC], dt.float32, tag="gamt", bufs=1)
            nc.sync.dma_start(gam_t[:], P[f"gam{l}"][0:1, 0:OC])
            bet_t = sb.tile([1, OC], dt.float32, tag="bett", bufs=1)
            nc.sync.dma_start(bet_t[:], P[f"bet{l}"][0:1, 0:OC])
            stats = sb.tile([1, 2 * OC], dt.float32, tag="stats", bufs=1)
            nc.vector.tensor_copy(stats[0:1, 0:OC], ps_s)
            nc.vector.tensor_copy(stats[0:1, OC : 2 * OC], ps_q)
            st_in = dram.tile([1, 2 * OC], dt.float32, tag="st_in")
            st_out = dram.tile([1, 2 * OC], dt.float32, tag=f"st_out{l}", addr_space="Shared", name=f"st_out{l}")
            nc.sync.dma_start(st_in[:], stats[:])
            nc.gpsimd.collective_compute(
                "AllReduce", ALU.add, ins=[st_in[:]], outs=[st_out[:]], replica_groups=[list(range(NC))]
            )
            st2 = sb.tile([1, 2 * OC], dt.float32, tag="st2", bufs=1)
            nc.sync.dma_start(st2[:], st_out[:])
            m = sb.tile([1, OC], dt.float32, tag="bn_m", bufs=1)
            q = sb.tile([1, OC], dt.float32, tag="bn_q", bufs=1)
            nc.vector.tensor_scalar(out=m[:], in0=st2[0:1, 0:OC], scalar1=1.0 / N_NODES, scalar2=None, op0=ALU.mult)
            nc.vector.tensor_scalar(out=q[:], in0=st2[0:1, OC : 2 * OC], scalar1=1.0 / N_NODES, scalar2=None, op0=ALU.mult)
            var = sb.tile([1, OC], dt.float32, tag="bn_v", bufs=1)
            nc.vector.tensor_tensor(out=var[:], in0=m[:], in1=m[:], op=ALU.mult)
            nc.vector.tensor_tensor(out=var[:], in0=q[:], in1=var[:], op=ALU.subtract)
            epsc = sb.tile([1, 1], dt.float32, tag="epsc")
            nc.vector.memset(epsc[:], EPS)
            sd = sb.tile([1, OC], dt.float32, tag="bn_sd", bufs=1)
            nc.scalar.activation(sd[:], var[:], ACTF.Sqrt, bias=epsc[0:1, 0:1])
            rs = sb.tile([1, OC], dt.float32, tag="bn_rs", bufs=1)
            nc.vector.reciprocal(rs[:], sd[:])
            s_row = sb.tile([1, OC], dt.float32, tag="bn_s", bufs=1)
            nc.vector.tensor_tensor(out=s_row[:], in0=rs[:], in1=gam_t[:], op=ALU.mult)
            b_row = sb.tile([1, OC], dt.float32, tag="bn_b", bufs=1)
            nc.vector.tensor_tensor(out=b_row[:], in0=m[:], in1=s_row[:], op=ALU.mult)
            nc.vector.tensor_tensor(out=b_row[:], in0=bet_t[:], in1=b_row[:], op=ALU.subtract)
            sbc = sb.tile([128, 2 * nch], dt.float32, tag="sbc")
            for k in range(nch):
                ps_c = ps.tile([128, 1], dt.float32, tag="small")
                nc.tensor.matmul(out=ps_c[:], lhsT=s_row[0:1, k * 128 : (k + 1) * 128], rhs=onesr_sb[0:1, 0:1], start=True, stop=True)
                nc.vector.tensor_copy(sbc[:, k : k + 1], ps_c[:])
                ps_c2 = ps.tile([128, 1], dt.float32, tag="small")
                nc.tensor.matmul(out=ps_c2[:], lhsT=b_row[0:1, k * 128 : (k + 1) * 128], rhs=onesr_sb[0:1, 0:1], start=True, stop=True)
                nc.vector.tensor_copy(sbc[:, nch + k : nch + k + 1], ps_c2[:])
            # affine + ELU in place on hTe, stripes of 4 blocks
            SW = 4
            for s0 in range(0, NBLK, SW):
                sw = min(SW, NBLK - s0)
                W = sw * 128
                for k in range(nch):
                    ystr = sb.tile([128, SW * 128], dt.bfloat16, tag="ystr", bufs=1)
                    nc.vector.tensor_scalar(
                        out=ystr[:, 0:W], in0=hTe[:, k * N2 + s0 * 128 : k * N2 + s0 * 128 + W],
                        scalar1=sbc[:, k : k + 1], scalar2=sbc[:, nch + k : nch + k + 1],
                        op0=ALU.mult, op1=ALU.add,
                    )
                    t1 = sb.tile([128, SW * 128], dt.bfloat16, tag="elu1", bufs=1)
                    nc.vector.tensor_scalar(out=t1[:, 0:W], in0=ystr[:, 0:W], scalar1=0.0, scalar2=None, op0=ALU.min)
                    e1 = sb.tile([128, SW * 128], dt.bfloat16, tag="elu2", bufs=1)
                    nc.scalar.activation(e1[:, 0:W], t1[:, 0:W], ACTF.Exp)
                    r1 = sb.tile([128, SW * 128], dt.bfloat16, tag="elu3", bufs=1)
                    nc.vector.tensor_scalar(out=r1[:, 0:W], in0=ystr[:, 0:W], scalar1=0.0, scalar2=-1.0, op0=ALU.max, op1=ALU.add)
                    nc.vector.tensor_tensor(
                        out=hTe[:, k * N2 + s0 * 128 : k * N2 + s0 * 128 + W],
                        in0=e1[:, 0:W], in1=r1[:, 0:W], op=ALU.add,
                    )

        for l in range(3):
            x_phase(l)
            ps_s = ps.tile([1, 512], dt.float32, tag="row", name="ps_s")
            ps_q = ps.tile([1, 512], dt.float32, tag="row", name="ps_q")
            ps_s = ps_s[:, 0 : LCFG[l][1]]
            ps_q = ps_q[:, 0 : LCFG[l][1]]
            p1 = make_pass1(l, ps_s, ps_q)
            edge_phase(l, p1)
            for kc in range(K_CH):
                p1(kc)
            post_phase(l, ps_s, ps_q)

        # =========================================================
        # pooling + classifier (baseline)
        # =========================================================
        meanT = sb1.tile([128, N_GRAPHS], dt.float32, tag="meanT")
        maxT = sb1.tile([128, N_GRAPHS], dt.float32, tag="maxT")
        nc.vector.memset(meanT[:], 0)
        nc.vector.memset(maxT[:], -1e30)
        mask8_sb = load_sb(i_mask8, [128, NC], dt.float32, "mask8")
        mask8n_sb = load_sb(i_mask8n, [128, NC], dt.float32, "mask8n")
        for cc in range(NC):
            scrm = sb.tile([128, N_GRAPHS], dt.float32, tag="scrm", bufs=1)
            scrx = sb.tile([128, N_GRAPHS], dt.float32, tag="scrx", bufs=1)
            nc.vector.memset(scrm[:], 0)
            nc.vector.memset(scrx[:], -1e30)
            for (a, bnd, g, inv) in segs[cc]:
                r1 = sb.tile([128, 1], dt.float32, tag="segr")
                nc.vector.tensor_reduce(out=r1[:], in_=hTe[:, a:bnd], axis=AXX, op=ALU.add)
                nc.vector.tensor_scalar(out=scrm[:, g : g + 1], in0=r1[:], scalar1=inv, scalar2=None, op0=ALU.mult)
                nc.vector.tensor_reduce(out=scrx[:, g : g + 1], in_=hTe[:, a:bnd], axis=AXX, op=ALU.max)
            nc.vector.tensor_scalar(out=scrm[:], in0=scrm[:], scalar1=mask8_sb[:, cc : cc + 1], scalar2=None, op0=ALU.mult)
            nc.vector.tensor_tensor(out=meanT[:], in0=meanT[:], in1=scrm[:], op=ALU.add)
            nc.vector.tensor_scalar(
                out=scrx[:], in0=scrx[:], scalar1=mask8_sb[:, cc : cc + 1],
                scalar2=mask8n_sb[:, cc : cc + 1], op0=ALU.mult, op1=ALU.add,
            )
            nc.vector.tensor_tensor(out=maxT[:], in0=maxT[:], in1=scrx[:], op=ALU.max)
        pm_in = dram.tile([128, N_GRAPHS], dt.float32, tag="pm_in")
        pm_out = dram.tile([128, N_GRAPHS], dt.float32, tag="pm_out", addr_space="Shared")
        px_in = dram.tile([128, N_GRAPHS], dt.float32, tag="px_in")
        px_out = dram.tile([128, N_GRAPHS], dt.float32, tag="px_out", addr_space="Shared")
        nc.sync.dma_start(pm_in[:], meanT[:])
        nc.sync.dma_start(px_in[:], maxT[:])
        nc.gpsimd.collective_compute("AllReduce", ALU.add, ins=[pm_in[:]], outs=[pm_out[:]], replica_groups=[list(range(NC))])
        nc.gpsimd.collective_compute("AllReduce", ALU.max, ins=[px_in[:]], outs=[px_out[:]], replica_groups=[list(range(NC))])
        meanF, maxF = meanT, maxT
        nc.sync.dma_start(meanF[:], pm_out[:])
        nc.sync.dma_start(maxF[:], px_out[:])

        cw1t_sb = [None, None]
        cw2t_sb = [None, None]
        for k in range(2):
            cw1t_sb[k] = sb1.tile([128, GDIM], dt.float32, tag=f"cw1t{k}", name=f"cw1t{k}")
            nc.sync.dma_start(cw1t_sb[k][:], P["cw1t"][k * 128 : (k + 1) * 128, :])
            cw2t_sb[k] = sb1.tile([128, NUM_CLASSES], dt.float32, tag=f"cw2t{k}", name=f"cw2t{k}")
            nc.sync.dma_start(cw2t_sb[k][:], P["cw2t"][k * 128 : (k + 1) * 128, :])
        cb1c_sb = load_sb(P["cb1c"], [128, 2], dt.float32, "cb1c")
        cb2c_sb = load_sb(P["cb2c"], [NUM_CLASSES, 1], dt.float32, "cb2c")
        hidT = sb1.tile([128, 2 * N_GRAPHS], dt.float32, tag="hidT")
        for hc in range(2):
            ps_h = ps.tile([128, N_GRAPHS], dt.float32, tag="med")
            for dc, embT in enumerate([meanF, maxF]):
                nc.tensor.matmul(
                    out=ps_h[:], lhsT=cw1t_sb[dc][:, hc * 128 : (hc + 1) * 128],
                    rhs=embT[:], start=(dc == 0), stop=(dc == 1),
                )
            nc.vector.tensor_scalar(
                out=hidT[:, hc * N_GRAPHS : (hc + 1) * N_GRAPHS], in0=ps_h[:],
                scalar1=cb1c_sb[:, hc : hc + 1], scalar2=0.0, op0=ALU.add, op1=ALU.max,
            )
        ps_o = ps.tile([NUM_CLASSES, N_GRAPHS], dt.float32, tag="med")
        for hc in range(2):
            nc.tensor.matmul(
                out=ps_o[:], lhsT=cw2t_sb[hc][:],
                rhs=hidT[:, hc * N_GRAPHS : (hc + 1) * N_GRAPHS], start=(hc == 0), stop=(hc == 1),
            )
        osb = sb1.tile([NUM_CLASSES, N_GRAPHS], dt.float32, tag="osb")
        nc.vector.tensor_scalar(out=osb[:], in0=ps_o[:], scalar1=cb2c_sb[:], scalar2=None, op0=ALU.add)
        for gc in range(2):
            ps_tt = ps.tile([128, NUM_CLASSES], dt.float32, tag="med")
            nc.tensor.matmul(
                out=ps_tt[:], lhsT=osb[:, gc * 128 : (gc + 1) * 128],
                rhs=idenf_sb[0:NUM_CLASSES, 0:NUM_CLASSES], start=True, stop=True,
            )
            ot = sb1.tile([128, NUM_CLASSES], dt.float32, tag="ot")
            nc.vector.tensor_copy(ot[:], ps_tt[:])
            nc.sync.dma_start(out_dram[gc * 128 : (gc + 1) * 128, :], ot[:])

    nc.compile()
    return nc


def kernel(**inputs):
    import concourse.bass_utils as bass_utils
    import hashlib

    pre = preprocess(inputs["x"], inputs["edge_index"], inputs["depth"], inputs["batch"])
    pb = build_param_blobs(inputs)

    sch = pre["sched"]
    sig = hashlib.sha1(
        sch["slots_w"].tobytes() + repr(sch["spans"]).encode() + repr(sch["segs"]).encode()
    ).hexdigest()
    if _CACHE.get("sig") != sig:
        _CACHE["built"] = build_nc(pre)
        _CACHE["sig"] = sig
    nc = _CACHE["built"]

    in_maps = []
    for c in range(NC):
        b = pre["blobs"][c]
        m = dict(
            xidx=b["xidx"], aldidx=b["aldidx"], dstloc=b["dstloc"],
            emb_idx=b["emb_idx"], depth_row=b["depth_row"],
            mask8=b["mask8"], mask8n=b["mask8n"],
        )
        m.update(pb)
        in_maps.append(m)

    import os, time

    trace = bool(int(os.environ.get("KERNEL_TRACE", "0")))
    t0 = time.time()
    res = bass_utils.run_bass_kernel_spmd(
        nc, in_maps, core_ids=list(range(NC)), trace=trace
    )
    _CACHE["run_s"] = time.time() - t0
    _CACHE["last_results"] = res
    return np.asarray(res.results[0]["out"], dtype=np.float32)


if __name__ == "__main__":
    sys.path.insert(0, "/root/problem")
    import reference

    inp = {k: np.asarray(v) for k, v in reference.setup_inputs().items()}
    got = kernel(**inp)
    exp = np.asarray(reference.reference(**inp))
    err = np.abs(got - exp).max() / (np.abs(exp).max() + 1e-30)
    print("Relative error:", err)
